# revision 9
# baseline (speedup 1.0000x reference)
"""GCN inference kernel (y = D^-1/2 A D^-1/2 (x @ W.T)) on 8 Trainium2 NeuronCores.

Strategy (full inputs in, full output out; sharded internally):
  - Destination nodes are sharded across the 8 cores (12500 dsts each);
    edges are owned by the core that owns their dst, so the segment-sum is
    core-local (per the sharding hint).
  - Phase A (replicated): every core computes the full scaled projection
    table h~[n] = (dinv[n]*x[n]) @ W.T in bf16 with PE matmuls (dinv is
    folded into x host-side) and writes it to per-bucket HBM tables; rows
    are 256B (64 bf16 features + 64 bf16 pad, never read).  No collective:
    phase B's bucket-b gathers start as soon as bucket b's rows land.
  - Phase B (per core): SWDGE dma_gather streams h~[src] rows (256B each)
    for the core's dst-sorted edge list into SBUF; a one-hot selection
    matrix B (built on DVE in bf16 from dst-local ids vs an iota row)
    turns the segment-sum into bf16 PE matmuls accumulated in PSUM per
    128-dst tile; a final per-dst dinv scale (Scalar engine) lands y.
    All gather indices / dst-lane ids are preloaded into SBUF once.
  - All data-dependent structure (edge sort, padding, gather indices,
    one-hot ids, uniform per-core slice schedule) is prepared host-side in
    numpy; the device program is identical on all 8 cores (SPMD), only the
    per-core input arrays differ.
"""

from dataclasses import dataclass, field

import numpy as np
import ml_dtypes

import concourse.bacc as bacc
import concourse.mybir as mybir
import concourse.tile as tile
from concourse.bass_utils import run_bass_kernel_spmd

P = 128  # SBUF partitions
FIN = 128
FOUT = 64
ROWB = 128  # padded table row width (bf16 -> 256B rows for dma_gather)

BF16 = ml_dtypes.bfloat16


@dataclass
class Prm:
    N: int = 100000  # nodes
    C: int = 8  # cores
    BKCAP: int = 25600  # table rows per gather bucket (int16 idx limit)
    SWD: int = 512  # dst nodes per superwindow (TPSW * P)
    S_CAP: int = 12  # max slices per dma_gather call (pipelining granularity)
    NS: int = field(init=False)  # dst shard size per core
    N2: int = field(init=False)  # padded node count (multiple of C*P)
    NG: int = field(init=False)  # total write groups (128 nodes each)
    NBK: int = field(init=False)  # gather buckets
    BSZ: list = field(init=False)  # rows per bucket
    GPB: list = field(init=False)  # groups per bucket
    TPSW: int = field(init=False)  # dst tiles per superwindow
    NSW: int = field(init=False)  # superwindows per core

    def __post_init__(self):
        assert self.BKCAP % P == 0 and self.BKCAP <= 32767
        assert self.SWD % P == 0
        assert self.N % self.C == 0
        self.NS = self.N // self.C
        blk = self.C * P
        self.N2 = ((self.N + blk - 1) // blk) * blk
        self.NG = self.N2 // P
        self.NBK = (self.N2 + self.BKCAP - 1) // self.BKCAP
        self.BSZ = [
            min(self.BKCAP, self.N2 - b * self.BKCAP) for b in range(self.NBK)
        ]
        self.GPB = [sz // P for sz in self.BSZ]
        self.TPSW = self.SWD // P
        self.NSW = (self.NS + self.SWD - 1) // self.SWD


def _wrap_idx(vals16):
    """[K] int16 (K % 128 == 0) -> [128, K//16] wrapped+replicated layout."""
    k = vals16.shape[0]
    w16 = vals16.reshape(k // 16, 16).T  # [16, K/16]
    return np.tile(w16, (8, 1))  # [128, K/16]


@dataclass
class CallMeta:
    sw: int
    bk: int
    k: int  # call index within its (sw, bk) section
    S: int  # slices in this call (one dma_gather per call)
    icol: int  # column offset into gidx array (8 * slice offset)
    scol: int  # column offset into dstl array (slice offset)


def _schedule(prm, n_sl_u):
    """Uniform (core-independent) schedule from the padded slice counts.

    Each (sw, bk) section is chunked into gather calls of <= S_CAP slices.
    Matmuls are emitted bucket-major per sw so PE starts as soon as bucket
    0's gather lands; each dst-tile t accumulates into its own PSUM tensor
    (accumulation groups stay open across buckets).
    Returns (calls, mms_by_sw).
    mms_by_sw[sw] = list of (bk, s_in_section, t, start, stop).
    """
    calls = []
    mms_by_sw = []
    icol = 0
    scol = 0
    for sw in range(prm.NSW):
        for bk in range(prm.NBK):
            nsl = sum(int(n_sl_u[sw][bk][t]) for t in range(prm.TPSW))
            for k, a in enumerate(range(0, nsl, prm.S_CAP)):
                S = min(prm.S_CAP, nsl - a)
                calls.append(CallMeta(sw, bk, k, S, icol, scol))
                icol += 8 * S
                scol += S
        mms = []
        seen = [0] * prm.TPSW
        tot = [
            sum(int(n_sl_u[sw][bk][t]) for bk in range(prm.NBK))
            for t in range(prm.TPSW)
        ]
        for bk in range(prm.NBK):
            s0 = 0
            for t in range(prm.TPSW):
                for _ in range(int(n_sl_u[sw][bk][t])):
                    mms.append(
                        (bk, s0, t, seen[t] == 0, seen[t] == tot[t] - 1)
                    )
                    seen[t] += 1
                    s0 += 1
        mms_by_sw.append(mms)
    return calls, mms_by_sw


def _host_prep(x, edge_index, W, prm):
    N, C, NS = prm.N, prm.C, prm.NS
    src = np.asarray(edge_index[0], dtype=np.int64).astype(np.int32)
    dst = np.asarray(edge_index[1], dtype=np.int64).astype(np.int32)
    x = np.asarray(x, dtype=np.float32)
    W = np.asarray(W, dtype=np.float32)

    deg = np.bincount(dst, minlength=N).astype(np.float64)
    dinv = np.where(deg > 0, 1.0 / np.sqrt(np.maximum(deg, 1.0)), 0.0).astype(
        np.float32
    )

    # table slot of node n is n itself
    bk_e = (src // prm.BKCAP).astype(np.int32)
    rel_e = (src % prm.BKCAP).astype(np.int16)

    # per-edge attributes
    core_e = dst // NS
    edl = dst - core_e * NS
    sw_e = edl // prm.SWD
    t_e = (edl % prm.SWD) // P
    q_e = (edl % P).astype(np.float32)

    # per-core sorted cell structure
    ncell = prm.NSW * prm.NBK * prm.TPSW
    counts = np.zeros((C, ncell), dtype=np.int64)
    percore = []
    for c in range(C):
        m = core_e == c
        order = np.lexsort((edl[m], t_e[m], bk_e[m], sw_e[m]))
        cell = (sw_e[m] * prm.NBK + bk_e[m]) * prm.TPSW + t_e[m]
        counts[c] = np.bincount(cell, minlength=ncell)
        percore.append(
            {
                "rel": rel_e[m][order],
                "q": q_e[m][order],
                "cell": cell[order],
            }
        )

    # uniform slice counts, >= 1 for in-range (sw, t) on bucket 0
    n_sl_u = np.zeros((prm.NSW, prm.NBK, prm.TPSW), dtype=np.int64)
    cmax = counts.max(axis=0).reshape(prm.NSW, prm.NBK, prm.TPSW)
    n_sl_u[:] = (cmax + P - 1) // P
    for sw in range(prm.NSW):
        ntile = min(prm.TPSW, max(0, -(-(NS - sw * prm.SWD) // P)))
        for t in range(ntile):
            if n_sl_u[sw, :, t].sum() == 0:
                n_sl_u[sw, 0, t] = 1

    calls, mms_by_sw = _schedule(prm, n_sl_u)
    icols = sum(8 * cm.S for cm in calls)
    scols = sum(cm.S for cm in calls)

    # slot offset (in slices) of each cell in the uniform stream
    cell_sl = n_sl_u.reshape(ncell)
    cell_off = np.zeros(ncell, dtype=np.int64)
    np.cumsum(cell_sl[:-1], out=cell_off[1:])
    S_total = int(cell_sl.sum())

    # fill per-core gather-index / dst-local arrays
    gidx_all = np.zeros((C, P, icols), dtype=np.int16)
    dstl_all = np.full((C, P, scols), -1.0, dtype=BF16)
    for c in range(C):
        pc = percore[c]
        ne = pc["cell"].shape[0]
        cc = counts[c]
        starts = np.zeros(ncell, dtype=np.int64)
        np.cumsum(cc[:-1], out=starts[1:])
        rank = np.arange(ne, dtype=np.int64) - starts[pc["cell"]]
        pos = cell_off[pc["cell"]] * P + rank  # global slot position
        vals = np.zeros(S_total * P, dtype=np.int16)
        dvals = np.full(S_total * P, -1.0, dtype=np.float32)
        vals[pos] = pc["rel"]
        dvals[pos] = pc["q"]
        for cm in calls:
            sl0 = cm.scol
            seg = vals[sl0 * P : (sl0 + cm.S) * P]
            gidx_all[c, :, cm.icol : cm.icol + 8 * cm.S] = _wrap_idx(seg)
            dstl_all[c, :, cm.scol : cm.scol + cm.S] = (
                dvals[sl0 * P : (sl0 + cm.S) * P].reshape(cm.S, P).T
            ).astype(BF16)

    # phase-A input: full dinv-scaled x, transposed, bf16 (same on all cores)
    xpad = np.zeros((prm.N2, FIN), dtype=np.float32)
    xpad[:N] = x * dinv[:, None]
    xT = np.ascontiguousarray(xpad.T).astype(BF16)  # [FIN, N2]
    WT = np.ascontiguousarray(W.T).astype(BF16)  # [FIN, FOUT]
    iota = (
        np.broadcast_to(np.arange(P, dtype=np.float32)[None, :], (P, P))
        .astype(BF16)
        .copy()
    )
    dinvD = np.zeros((C, P, prm.NSW * prm.TPSW), dtype=np.float32)
    w_idx = np.arange(prm.NSW * prm.TPSW)
    for c in range(C):
        node = c * NS + w_idx[:, None] * P + np.arange(P)[None, :]
        ok = node < (c + 1) * NS
        dv = np.where(ok, dinv[np.minimum(node, N - 1)], 0.0)
        dinvD[c][np.arange(P)[None, :], w_idx[:, None]] = dv

    inputs = []
    for c in range(C):
        inputs.append(
            {
                "xT": xT,
                "WT": WT,
                "iota": iota,
                "dinvD": dinvD[c],
                "gidx": gidx_all[c],
                "dstl": dstl_all[c],
            }
        )
    return inputs, calls, mms_by_sw, icols, scols


def _split_sync_waits(nc):
    """This env's walrus rejects >1 sync wait on some opcodes; keep 1 wait
    per instruction, moving extras onto preceding same-engine NOPs."""
    for bb in nc.main_func.blocks:
        insts = bb.instructions
        i = 0
        while i < len(insts):
            ins = insts[i]
            si = ins.sync_info
            if si is not None and si.on_wait is not None and len(si.on_wait) > 1:
                waits = list(si.on_wait)
                keep, extra = waits[-1:], waits[:-1]
                k = 0
                while extra:
                    chunk, extra = extra[:1], extra[1:]
                    nop = mybir.InstNoOp(name=f"{ins.name}-ws{k}", ins=[], outs=[])
                    nop.engine = ins.engine
                    nop.sync_info = mybir.SyncInfo(on_wait=chunk, on_update=[])
                    nc.register_instruction(nop)
                    insts.insert(i, nop)
                    i += 1
                    k += 1
                ins.sync_info = mybir.SyncInfo(
                    on_wait=keep, on_update=list(si.on_update or [])
                )
            i += 1


def _build_program(prm, calls, mms_by_sw, icols, scols):
    f32 = mybir.dt.float32
    bf16 = mybir.dt.bfloat16
    nc = bacc.Bacc("TRN2", num_swdge_queues=4)

    xT = nc.declare_dram_parameter("xT", [FIN, prm.N2], bf16, isOutput=False)
    WT = nc.declare_dram_parameter("WT", [FIN, FOUT], bf16, isOutput=False)
    iota = nc.declare_dram_parameter("iota", [P, P], bf16, isOutput=False)
    dinvD = nc.declare_dram_parameter(
        "dinvD", [P, prm.NSW * prm.TPSW], f32, isOutput=False
    )
    gidx = nc.declare_dram_parameter(
        "gidx", [P, icols], mybir.dt.int16, isOutput=False
    )
    dstl = nc.declare_dram_parameter("dstl", [P, scols], bf16, isOutput=False)
    y = nc.declare_dram_parameter("y", [prm.NS, FOUT], f32, isOutput=True)
    # the full table, replicated per core, split per gather bucket so phase
    # B's bucket-b gathers only wait on bucket b's writes
    TBL = [
        nc.dram_tensor(f"tbl{b}", [prm.BSZ[b], ROWB], bf16)
        for b in range(prm.NBK)
    ]

    with tile.TileContext(nc) as tc:
        with tc.tile_pool(name="const", bufs=1) as cpool:
            wt_sb = cpool.tile([FIN, FOUT], bf16, tag="wt")
            nc.sync.dma_start(out=wt_sb[:], in_=WT[:])
            iota_sb = cpool.tile([P, P], bf16, tag="io")
            nc.sync.dma_start(out=iota_sb[:], in_=iota[:])
            dinvD_sb = cpool.tile([P, prm.NSW * prm.TPSW], f32, tag="dd")
            nc.sync.dma_start(out=dinvD_sb[:], in_=dinvD[:])
            # preload all gather indices / dst-lane ids (removes per-call
            # index DMAs and their sequencer + dependency cost entirely)
            gidx_sb = cpool.tile([P, icols], mybir.dt.int16, tag="gi")
            nc.scalar.dma_start(out=gidx_sb[:], in_=gidx[:])
            dstl_sb = cpool.tile([P, scols], bf16, tag="dl")
            nc.scalar.dma_start(out=dstl_sb[:], in_=dstl[:])

            # ---------------- Phase A: build the full h~ table ------------
            GB = 8  # groups per xT load block
            QG = 4  # groups per PSUM tile / activation / table write
            assert prm.NG % GB == 0 and GB % QG == 0
            for gpb in prm.GPB:
                assert gpb % QG == 0
            with (
                tc.tile_pool(name="pa", bufs=3) as pa,
                tc.tile_pool(name="pat", bufs=4) as pat,
                tc.tile_pool(name="psa", bufs=4, space="PSUM") as psa,
            ):
                for blk in range(prm.NG // GB):
                    xt = pa.tile([P, GB * P], bf16, tag="xt")
                    nc.sync.dma_start(
                        out=xt[:], in_=xT[:, blk * GB * P : (blk + 1) * GB * P]
                    )
                    for q in range(GB // QG):
                        g0 = blk * GB + q * QG  # first group of this quad
                        hps = psa.tile([P, QG, FOUT], f32, tag="hps")
                        for j in range(QG):
                            nc.tensor.matmul(
                                out=hps[:, j, :],
                                lhsT=xt[:, (q * QG + j) * P : (q * QG + j + 1) * P],
                                rhs=wt_sb[:],
                                start=True,
                                stop=True,
                            )
                        tsb = pat.tile([P, QG, ROWB], bf16, tag="tsb")
                        nc.scalar.activation(
                            out=tsb[:, :, :FOUT],
                            in_=hps[:],
                            func=mybir.ActivationFunctionType.Copy,
                        )
                        b = (g0 * P) // prm.BKCAP
                        r0 = g0 * P - b * prm.BKCAP
                        nc.sync.dma_start(
                            out=TBL[b][r0 : r0 + QG * P, :].rearrange(
                                "(j p) f -> p j f", p=P
                            ),
                            in_=tsb[:],
                        )

            # ---------------- Phase B: gather + segment-sum ----------------
            S_MAX = max((cm.S for cm in calls), default=1)
            calls_by_sw = [[] for _ in range(prm.NSW)]
            for cm in calls:
                calls_by_sw[cm.sw].append(cm)
            qctr = [0]
            with (
                tc.tile_pool(name="pg", bufs=20) as pg,
                tc.tile_pool(name="pb", bufs=8) as pb,
                tc.tile_pool(name="py", bufs=2) as py,
                tc.tile_pool(name="psb", bufs=2, space="PSUM") as psb,
            ):
                for sw in range(prm.NSW):
                    if not calls_by_sw[sw]:
                        continue
                    tiles = {}  # (bk, k) -> (g_t, b_t)
                    for cm in calls_by_sw[sw]:
                        S = cm.S
                        g_t = pg.tile([P, S_MAX, ROWB], bf16, tag="g")
                        nc.gpsimd.dma_gather(
                            out_ap=g_t[:, :S, :],
                            in_ap=TBL[cm.bk][:],
                            idxs_ap=gidx_sb[:, cm.icol : cm.icol + 8 * S],
                            num_idxs=S * P,
                            num_idxs_reg=S * P,
                            elem_size=ROWB,
                            single_packet=False,
                            queue_num=qctr[0] % 4,
                        )
                        qctr[0] += 1
                        b_t = pb.tile([P, S_MAX, P], bf16, tag="b")
                        nc.vector.tensor_tensor(
                            out=b_t[:, :S, :],
                            in0=dstl_sb[:, cm.scol : cm.scol + S][
                                :, :, None
                            ].to_broadcast([P, S, P]),
                            in1=iota_sb[:, None, :].to_broadcast([P, S, P]),
                            op=mybir.AluOpType.is_equal,
                        )
                        tiles[(cm.bk, cm.k)] = (g_t, b_t)
                    psum_t = [
                        psb.tile([P, FOUT], f32, tag=f"acc{t}", name=f"acc{t}")
                        for t in range(prm.TPSW)
                    ]
                    for bk, s, t, st, sp in mms_by_sw[sw]:
                        g_t, b_t = tiles[(bk, s // prm.S_CAP)]
                        sl = s % prm.S_CAP
                        nc.tensor.matmul(
                            out=psum_t[t][:],
                            lhsT=b_t[:, sl, :],
                            rhs=g_t[:, sl, :FOUT],
                            start=st,
                            stop=sp,
                        )
                    # scale by dinv[dst] on the otherwise-idle Scalar engine
                    rows_sw = min(prm.SWD, prm.NS - sw * prm.SWD)
                    nt = (rows_sw + P - 1) // P  # valid dst tiles this sw
                    ysb = py.tile([P, prm.TPSW, FOUT], f32, tag="ysb")
                    for t in range(nt):
                        w = sw * prm.TPSW + t
                        nc.scalar.activation(
                            out=ysb[:, t, :],
                            in_=psum_t[t][:],
                            func=mybir.ActivationFunctionType.Copy,
                            scale=dinvD_sb[:, w : w + 1],
                        )
                    for t in range(nt):
                        rt = min(P, rows_sw - t * P)
                        r0 = sw * prm.SWD + t * P
                        nc.sync.dma_start(out=y[r0 : r0 + rt, :], in_=ysb[:rt, t, :])

    nc.compile()
    _split_sync_waits(nc)
    return nc


def _get_program_and_prep(x, edge_index, W, prm):
    inputs, calls, mms_by_sw, icols, scols = _host_prep(x, edge_index, W, prm)
    nc = _build_program(prm, calls, mms_by_sw, icols, scols)
    return nc, inputs


def kernel(x, edge_index, W):
    prm = Prm(N=int(x.shape[0]))
    nc, inputs = _get_program_and_prep(x, edge_index, W, prm)
    res = run_bass_kernel_spmd(nc, inputs, list(range(prm.C)))
    y = np.concatenate([res.results[c]["y"] for c in range(prm.C)], axis=0)
    return y.astype(np.float32)


def run_with_trace(x, edge_index, W, trace_cores=None):
    """test.py helper: returns (y, BassKernelResults) with profiling."""
    prm = Prm(N=int(x.shape[0]))
    nc, inputs = _get_program_and_prep(x, edge_index, W, prm)
    res = run_bass_kernel_spmd(
        nc, inputs, list(range(prm.C)), trace=True, trace_cores=trace_cores
    )
    y = np.concatenate([res.results[c]["y"] for c in range(prm.C)], axis=0)
    return y.astype(np.float32), res


# revision 12
# speedup vs baseline: 1.0203x; 1.0203x over previous
"""GCN inference kernel (y = D^-1/2 A D^-1/2 (x @ W.T)) on 8 Trainium2 NeuronCores.

Strategy (full inputs in, full output out; sharded internally):
  - Destination nodes are sharded across the 8 cores (12500 dsts each);
    edges are owned by the core that owns their dst, so the segment-sum is
    core-local (per the sharding hint).
  - Phase A (replicated): every core computes the full scaled projection
    table h~[n] = (dinv[n]*x[n]) @ W.T in bf16 with PE matmuls (dinv is
    folded into x host-side) and writes it to per-bucket HBM tables; rows
    are 256B (64 bf16 features + 64 bf16 pad, never read).  No collective:
    phase B's bucket-b gathers start as soon as bucket b's rows land.
  - Phase B (per core): SWDGE dma_gather streams h~[src] rows (256B each)
    for the core's dst-sorted edge list into SBUF; a one-hot selection
    matrix B (built on DVE in bf16 from dst-local ids vs an iota row)
    turns the segment-sum into bf16 PE matmuls accumulated in PSUM per
    128-dst tile; a final per-dst dinv scale (Scalar engine) lands y.
    All gather indices / dst-lane ids are preloaded into SBUF once.
  - All data-dependent structure (edge sort, padding, gather indices,
    one-hot ids, uniform per-core slice schedule) is prepared host-side in
    numpy; the device program is identical on all 8 cores (SPMD), only the
    per-core input arrays differ.
"""

from dataclasses import dataclass, field

import numpy as np
import ml_dtypes

import concourse.bacc as bacc
import concourse.mybir as mybir
import concourse.tile as tile
from concourse.bass_utils import run_bass_kernel_spmd

P = 128  # SBUF partitions
FIN = 128
FOUT = 64
ROWB = 128  # padded table row width (bf16 -> 256B rows for dma_gather)

BF16 = ml_dtypes.bfloat16


@dataclass
class Prm:
    N: int = 100000  # nodes
    C: int = 8  # cores
    BKCAP: int = 25600  # table rows per gather bucket (int16 idx limit)
    SWD: int = 512  # dst nodes per superwindow (TPSW * P)
    S_CAP: int = 12  # max slices per dma_gather call (pipelining granularity)
    NS: int = field(init=False)  # dst shard size per core
    N2: int = field(init=False)  # padded node count (multiple of C*P)
    NG: int = field(init=False)  # total write groups (128 nodes each)
    NBK: int = field(init=False)  # gather buckets
    BSZ: list = field(init=False)  # rows per bucket
    GPB: list = field(init=False)  # groups per bucket
    TPSW: int = field(init=False)  # dst tiles per superwindow
    NSW: int = field(init=False)  # superwindows per core

    def __post_init__(self):
        assert self.BKCAP % P == 0 and self.BKCAP <= 32767
        assert self.SWD % P == 0
        assert self.N % self.C == 0
        self.NS = self.N // self.C
        blk = self.C * P
        self.N2 = ((self.N + blk - 1) // blk) * blk
        self.NG = self.N2 // P
        self.NBK = (self.N2 + self.BKCAP - 1) // self.BKCAP
        self.BSZ = [
            min(self.BKCAP, self.N2 - b * self.BKCAP) for b in range(self.NBK)
        ]
        self.GPB = [sz // P for sz in self.BSZ]
        self.TPSW = self.SWD // P
        self.NSW = (self.NS + self.SWD - 1) // self.SWD


def _wrap_idx(vals16):
    """[K] int16 (K % 128 == 0) -> [128, K//16] wrapped+replicated layout."""
    k = vals16.shape[0]
    w16 = vals16.reshape(k // 16, 16).T  # [16, K/16]
    return np.tile(w16, (8, 1))  # [128, K/16]


@dataclass
class CallMeta:
    sw: int
    bk: int
    k: int  # call index within its (sw, bk) section
    S: int  # slices in this call (one dma_gather per call)
    icol: int  # column offset into gidx array (8 * slice offset)
    scol: int  # column offset into dstl array (slice offset)


def _schedule(prm, n_sl_u):
    """Uniform (core-independent) schedule from the padded slice counts.

    Each (sw, bk) section is chunked into gather calls of <= S_CAP slices.
    Matmuls are emitted bucket-major per sw so PE starts as soon as bucket
    0's gather lands; each dst-tile t accumulates into its own PSUM tensor
    (accumulation groups stay open across buckets).
    Returns (calls, mms_by_sw).
    mms_by_sw[sw] = list of (bk, s_in_section, t, start, stop).
    """
    calls = []
    mms_by_sw = []
    icol = 0
    scol = 0
    for sw in range(prm.NSW):
        for bk in range(prm.NBK):
            nsl = sum(int(n_sl_u[sw][bk][t]) for t in range(prm.TPSW))
            for k, a in enumerate(range(0, nsl, prm.S_CAP)):
                S = min(prm.S_CAP, nsl - a)
                calls.append(CallMeta(sw, bk, k, S, icol, scol))
                icol += 8 * S
                scol += S
        mms = []
        seen = [0] * prm.TPSW
        tot = [
            sum(int(n_sl_u[sw][bk][t]) for bk in range(prm.NBK))
            for t in range(prm.TPSW)
        ]
        for bk in range(prm.NBK):
            s0 = 0
            for t in range(prm.TPSW):
                for _ in range(int(n_sl_u[sw][bk][t])):
                    mms.append(
                        (bk, s0, t, seen[t] == 0, seen[t] == tot[t] - 1)
                    )
                    seen[t] += 1
                    s0 += 1
        mms_by_sw.append(mms)
    return calls, mms_by_sw


def _host_prep(x, edge_index, W, prm):
    N, C, NS = prm.N, prm.C, prm.NS
    src = np.asarray(edge_index[0], dtype=np.int64).astype(np.int32)
    dst = np.asarray(edge_index[1], dtype=np.int64).astype(np.int32)
    x = np.asarray(x, dtype=np.float32)
    W = np.asarray(W, dtype=np.float32)

    deg = np.bincount(dst, minlength=N).astype(np.float64)
    dinv = np.where(deg > 0, 1.0 / np.sqrt(np.maximum(deg, 1.0)), 0.0).astype(
        np.float32
    )

    # table slot of node n: groups are written in blocks of QG=8 with the
    # partition dim outermost so phase-A table writes are contiguous 2KB
    # per partition: slot = (g//8)*1024 + (n%128)*8 + g%8,  g = n//128
    g_e = src // P
    p_e = src % P
    slot_e = (g_e // 8) * (8 * P) + p_e * 8 + (g_e % 8)
    bk_e = (slot_e // prm.BKCAP).astype(np.int32)
    rel_e = (slot_e % prm.BKCAP).astype(np.int16)

    # per-edge attributes
    core_e = dst // NS
    edl = dst - core_e * NS
    sw_e = edl // prm.SWD
    t_e = (edl % prm.SWD) // P
    q_e = (edl % P).astype(np.float32)

    # per-core sorted cell structure
    ncell = prm.NSW * prm.NBK * prm.TPSW
    counts = np.zeros((C, ncell), dtype=np.int64)
    percore = []
    for c in range(C):
        m = core_e == c
        order = np.lexsort((edl[m], t_e[m], bk_e[m], sw_e[m]))
        cell = (sw_e[m] * prm.NBK + bk_e[m]) * prm.TPSW + t_e[m]
        counts[c] = np.bincount(cell, minlength=ncell)
        percore.append(
            {
                "rel": rel_e[m][order],
                "q": q_e[m][order],
                "cell": cell[order],
            }
        )

    # uniform slice counts, >= 1 for in-range (sw, t) on bucket 0
    n_sl_u = np.zeros((prm.NSW, prm.NBK, prm.TPSW), dtype=np.int64)
    cmax = counts.max(axis=0).reshape(prm.NSW, prm.NBK, prm.TPSW)
    n_sl_u[:] = (cmax + P - 1) // P
    for sw in range(prm.NSW):
        ntile = min(prm.TPSW, max(0, -(-(NS - sw * prm.SWD) // P)))
        for t in range(ntile):
            if n_sl_u[sw, :, t].sum() == 0:
                n_sl_u[sw, 0, t] = 1

    calls, mms_by_sw = _schedule(prm, n_sl_u)
    icols = sum(8 * cm.S for cm in calls)
    scols = sum(cm.S for cm in calls)

    # slot offset (in slices) of each cell in the uniform stream
    cell_sl = n_sl_u.reshape(ncell)
    cell_off = np.zeros(ncell, dtype=np.int64)
    np.cumsum(cell_sl[:-1], out=cell_off[1:])
    S_total = int(cell_sl.sum())

    # fill per-core gather-index / dst-local arrays
    gidx_all = np.zeros((C, P, icols), dtype=np.int16)
    dstl_all = np.full((C, P, scols), -1.0, dtype=BF16)
    for c in range(C):
        pc = percore[c]
        ne = pc["cell"].shape[0]
        cc = counts[c]
        starts = np.zeros(ncell, dtype=np.int64)
        np.cumsum(cc[:-1], out=starts[1:])
        rank = np.arange(ne, dtype=np.int64) - starts[pc["cell"]]
        pos = cell_off[pc["cell"]] * P + rank  # global slot position
        vals = np.zeros(S_total * P, dtype=np.int16)
        dvals = np.full(S_total * P, -1.0, dtype=np.float32)
        vals[pos] = pc["rel"]
        dvals[pos] = pc["q"]
        for cm in calls:
            sl0 = cm.scol
            seg = vals[sl0 * P : (sl0 + cm.S) * P]
            gidx_all[c, :, cm.icol : cm.icol + 8 * cm.S] = _wrap_idx(seg)
            dstl_all[c, :, cm.scol : cm.scol + cm.S] = (
                dvals[sl0 * P : (sl0 + cm.S) * P].reshape(cm.S, P).T
            ).astype(BF16)

    # phase-A input: full dinv-scaled x, transposed, bf16 (same on all cores)
    xpad = np.zeros((prm.N2, FIN), dtype=np.float32)
    xpad[:N] = x * dinv[:, None]
    xT = np.ascontiguousarray(xpad.T).astype(BF16)  # [FIN, N2]
    WT = np.ascontiguousarray(W.T).astype(BF16)  # [FIN, FOUT]
    iota = (
        np.broadcast_to(np.arange(P, dtype=np.float32)[None, :], (P, P))
        .astype(BF16)
        .copy()
    )
    dinvD = np.zeros((C, P, prm.NSW * prm.TPSW), dtype=np.float32)
    w_idx = np.arange(prm.NSW * prm.TPSW)
    for c in range(C):
        node = c * NS + w_idx[:, None] * P + np.arange(P)[None, :]
        ok = node < (c + 1) * NS
        dv = np.where(ok, dinv[np.minimum(node, N - 1)], 0.0)
        dinvD[c][np.arange(P)[None, :], w_idx[:, None]] = dv

    inputs = []
    for c in range(C):
        inputs.append(
            {
                "xT": xT,
                "WT": WT,
                "iota": iota,
                "dinvD": dinvD[c],
                "gidx": gidx_all[c],
                "dstl": dstl_all[c],
            }
        )
    return inputs, calls, mms_by_sw, icols, scols


def _split_sync_waits(nc):
    """This env's walrus rejects >1 sync wait on some opcodes; keep 1 wait
    per instruction, moving extras onto preceding same-engine NOPs."""
    for bb in nc.main_func.blocks:
        insts = bb.instructions
        i = 0
        while i < len(insts):
            ins = insts[i]
            si = ins.sync_info
            if si is not None and si.on_wait is not None and len(si.on_wait) > 1:
                waits = list(si.on_wait)
                keep, extra = waits[-1:], waits[:-1]
                k = 0
                while extra:
                    chunk, extra = extra[:1], extra[1:]
                    nop = mybir.InstNoOp(name=f"{ins.name}-ws{k}", ins=[], outs=[])
                    nop.engine = ins.engine
                    nop.sync_info = mybir.SyncInfo(on_wait=chunk, on_update=[])
                    nc.register_instruction(nop)
                    insts.insert(i, nop)
                    i += 1
                    k += 1
                ins.sync_info = mybir.SyncInfo(
                    on_wait=keep, on_update=list(si.on_update or [])
                )
            i += 1


def _build_program(prm, calls, mms_by_sw, icols, scols):
    f32 = mybir.dt.float32
    bf16 = mybir.dt.bfloat16
    nc = bacc.Bacc("TRN2", num_swdge_queues=4)

    xT = nc.declare_dram_parameter("xT", [FIN, prm.N2], bf16, isOutput=False)
    WT = nc.declare_dram_parameter("WT", [FIN, FOUT], bf16, isOutput=False)
    iota = nc.declare_dram_parameter("iota", [P, P], bf16, isOutput=False)
    dinvD = nc.declare_dram_parameter(
        "dinvD", [P, prm.NSW * prm.TPSW], f32, isOutput=False
    )
    gidx = nc.declare_dram_parameter(
        "gidx", [P, icols], mybir.dt.int16, isOutput=False
    )
    dstl = nc.declare_dram_parameter("dstl", [P, scols], bf16, isOutput=False)
    y = nc.declare_dram_parameter("y", [prm.NS, FOUT], f32, isOutput=True)
    # the full table, replicated per core, split per gather bucket so phase
    # B's bucket-b gathers only wait on bucket b's writes
    TBL = [
        nc.dram_tensor(f"tbl{b}", [prm.BSZ[b], ROWB], bf16)
        for b in range(prm.NBK)
    ]

    with tile.TileContext(nc) as tc:
        with tc.tile_pool(name="const", bufs=1) as cpool:
            wt_sb = cpool.tile([FIN, FOUT], bf16, tag="wt")
            nc.sync.dma_start(out=wt_sb[:], in_=WT[:])
            iota_sb = cpool.tile([P, P], bf16, tag="io")
            nc.sync.dma_start(out=iota_sb[:], in_=iota[:])
            dinvD_sb = cpool.tile([P, prm.NSW * prm.TPSW], f32, tag="dd")
            nc.sync.dma_start(out=dinvD_sb[:], in_=dinvD[:])
            # preload all gather indices / dst-lane ids (removes per-call
            # index DMAs and their sequencer + dependency cost entirely)
            gidx_sb = cpool.tile([P, icols], mybir.dt.int16, tag="gi")
            nc.scalar.dma_start(out=gidx_sb[:], in_=gidx[:])
            dstl_sb = cpool.tile([P, scols], bf16, tag="dl")
            nc.scalar.dma_start(out=dstl_sb[:], in_=dstl[:])

            # ---------------- Phase A: build the full h~ table ------------
            QG = 8  # groups per load / PSUM tile / activation / table write
            assert prm.NG % QG == 0
            for gpb in prm.GPB:
                assert gpb % QG == 0
            with (
                tc.tile_pool(name="pa", bufs=3) as pa,
                tc.tile_pool(name="pat", bufs=4) as pat,
                tc.tile_pool(name="psa", bufs=4, space="PSUM") as psa,
            ):
                for blk in range(prm.NG // QG):
                    xt = pa.tile([P, QG * P], bf16, tag="xt")
                    nc.scalar.dma_start(
                        out=xt[:], in_=xT[:, blk * QG * P : (blk + 1) * QG * P]
                    )
                    hps = psa.tile([P, QG, FOUT], f32, tag="hps")
                    for j in range(QG):
                        nc.tensor.matmul(
                            out=hps[:, j, :],
                            lhsT=xt[:, j * P : (j + 1) * P],
                            rhs=wt_sb[:],
                            start=True,
                            stop=True,
                        )
                    tsb = pat.tile([P, QG, ROWB], bf16, tag="tsb")
                    nc.scalar.activation(
                        out=tsb[:, :, :FOUT],
                        in_=hps[:],
                        func=mybir.ActivationFunctionType.Copy,
                    )
                    # block rows are laid out (p, j): partition-major, so the
                    # write is 2KB contiguous per partition
                    b = (blk * QG * P) // prm.BKCAP
                    r0 = blk * QG * P - b * prm.BKCAP
                    nc.sync.dma_start(
                        out=TBL[b][r0 : r0 + QG * P, :].rearrange(
                            "(p j) f -> p j f", j=QG
                        ),
                        in_=tsb[:],
                    )

            # ---------------- Phase B: gather + segment-sum ----------------
            S_MAX = max((cm.S for cm in calls), default=1)
            calls_by_sw = [[] for _ in range(prm.NSW)]
            for cm in calls:
                calls_by_sw[cm.sw].append(cm)
            # slices per (sw, bk) section and its slice/scol offsets
            sec_nsl = {}
            sec_scol = {}
            for cm in calls:
                key = (cm.sw, cm.bk)
                if key not in sec_nsl:
                    sec_nsl[key] = 0
                    sec_scol[key] = cm.scol
                sec_nsl[key] += cm.S
            SB_MAX = max(sec_nsl.values(), default=1)
            qctr = [0]
            with (
                tc.tile_pool(name="pg", bufs=20) as pg,
                tc.tile_pool(name="pb", bufs=4) as pb,
                tc.tile_pool(name="py", bufs=2) as py,
                tc.tile_pool(name="psb", bufs=2, space="PSUM") as psb,
            ):
                for sw in range(prm.NSW):
                    if not calls_by_sw[sw]:
                        continue
                    tiles = {}  # (bk, k) -> g_t
                    bsec = {}  # bk -> b_t (whole section, built off-chain)
                    for cm in calls_by_sw[sw]:
                        S = cm.S
                        g_t = pg.tile([P, S_MAX, ROWB], bf16, tag="g")
                        nc.gpsimd.dma_gather(
                            out_ap=g_t[:, :S, :],
                            in_ap=TBL[cm.bk][:],
                            idxs_ap=gidx_sb[:, cm.icol : cm.icol + 8 * S],
                            num_idxs=S * P,
                            num_idxs_reg=S * P,
                            elem_size=ROWB,
                            single_packet=False,
                            queue_num=qctr[0] % 4,
                        )
                        qctr[0] += 1
                        tiles[(cm.bk, cm.k)] = g_t
                        if (sw, cm.bk) in sec_nsl and cm.bk not in bsec:
                            ns = sec_nsl[(sw, cm.bk)]
                            sc = sec_scol[(sw, cm.bk)]
                            b_t = pb.tile([P, SB_MAX, P], bf16, tag="b")
                            nc.vector.tensor_tensor(
                                out=b_t[:, :ns, :],
                                in0=dstl_sb[:, sc : sc + ns][
                                    :, :, None
                                ].to_broadcast([P, ns, P]),
                                in1=iota_sb[:, None, :].to_broadcast([P, ns, P]),
                                op=mybir.AluOpType.is_equal,
                            )
                            bsec[cm.bk] = b_t
                    psum_t = [
                        psb.tile([P, FOUT], f32, tag=f"acc{t}", name=f"acc{t}")
                        for t in range(prm.TPSW)
                    ]
                    for bk, s, t, st, sp in mms_by_sw[sw]:
                        g_t = tiles[(bk, s // prm.S_CAP)]
                        sl = s % prm.S_CAP
                        nc.tensor.matmul(
                            out=psum_t[t][:],
                            lhsT=bsec[bk][:, s, :],
                            rhs=g_t[:, sl, :FOUT],
                            start=st,
                            stop=sp,
                        )
                    # scale by dinv[dst] on the otherwise-idle Scalar engine
                    rows_sw = min(prm.SWD, prm.NS - sw * prm.SWD)
                    nt = (rows_sw + P - 1) // P  # valid dst tiles this sw
                    ysb = py.tile([P, prm.TPSW, FOUT], f32, tag="ysb")
                    for t in range(nt):
                        w = sw * prm.TPSW + t
                        nc.scalar.activation(
                            out=ysb[:, t, :],
                            in_=psum_t[t][:],
                            func=mybir.ActivationFunctionType.Copy,
                            scale=dinvD_sb[:, w : w + 1],
                        )
                    for t in range(nt):
                        rt = min(P, rows_sw - t * P)
                        r0 = sw * prm.SWD + t * P
                        nc.sync.dma_start(out=y[r0 : r0 + rt, :], in_=ysb[:rt, t, :])

    nc.compile()
    _split_sync_waits(nc)
    return nc


def _get_program_and_prep(x, edge_index, W, prm):
    inputs, calls, mms_by_sw, icols, scols = _host_prep(x, edge_index, W, prm)
    nc = _build_program(prm, calls, mms_by_sw, icols, scols)
    return nc, inputs


def kernel(x, edge_index, W):
    prm = Prm(N=int(x.shape[0]))
    nc, inputs = _get_program_and_prep(x, edge_index, W, prm)
    res = run_bass_kernel_spmd(nc, inputs, list(range(prm.C)))
    y = np.concatenate([res.results[c]["y"] for c in range(prm.C)], axis=0)
    return y.astype(np.float32)


def run_with_trace(x, edge_index, W, trace_cores=None):
    """test.py helper: returns (y, BassKernelResults) with profiling."""
    prm = Prm(N=int(x.shape[0]))
    nc, inputs = _get_program_and_prep(x, edge_index, W, prm)
    res = run_bass_kernel_spmd(
        nc, inputs, list(range(prm.C)), trace=True, trace_cores=trace_cores
    )
    y = np.concatenate([res.results[c]["y"] for c in range(prm.C)], axis=0)
    return y.astype(np.float32), res


# revision 21
# speedup vs baseline: 1.0556x; 1.0346x over previous
"""GCN inference kernel (y = D^-1/2 A D^-1/2 (x @ W.T)) on 8 Trainium2 NeuronCores.

Strategy (full inputs in, full output out; sharded internally):
  - Destination nodes are sharded across the 8 cores (12500 dsts each);
    edges are owned by the core that owns their dst, so the segment-sum is
    core-local (per the sharding hint).
  - Phase A (replicated): every core computes the full scaled projection
    table h~[n] = (dinv[n]*x[n]) @ W.T in bf16 with PE matmuls (dinv is
    folded into x host-side) and writes it to per-bucket HBM tables; rows
    are 256B (64 bf16 features + 64 bf16 pad, never read).  No collective:
    phase B's bucket-b gathers start as soon as bucket b's rows land.
  - Phase B (per core): SWDGE dma_gather streams h~[src] rows (256B each)
    for the core's dst-sorted edge list into SBUF; a one-hot selection
    matrix B (built on DVE in bf16 from dst-local ids vs an iota row)
    turns the segment-sum into bf16 PE matmuls accumulated in PSUM per
    128-dst tile; a final per-dst dinv scale (Scalar engine) lands y.
    All gather indices / dst-lane ids are preloaded into SBUF once.
  - All data-dependent structure (edge sort, padding, gather indices,
    one-hot ids, uniform per-core slice schedule) is prepared host-side in
    numpy; the device program is identical on all 8 cores (SPMD), only the
    per-core input arrays differ.
"""

from dataclasses import dataclass, field

import numpy as np
import ml_dtypes

import concourse.bacc as bacc
import concourse.mybir as mybir
import concourse.tile as tile
from concourse.bass_utils import run_bass_kernel_spmd

P = 128  # SBUF partitions
FIN = 128
FOUT = 64
ROWB = 128  # padded table row width (bf16 -> 256B rows for dma_gather)

BF16 = ml_dtypes.bfloat16


@dataclass
class Prm:
    N: int = 100000  # nodes
    C: int = 8  # cores
    BKCAP: int = 25600  # table rows per gather bucket (int16 idx limit)
    SWD: int = 512  # dst nodes per superwindow (TPSW * P)
    S_CAP: int = 12  # max slices per dma_gather call (pipelining granularity)
    NS: int = field(init=False)  # dst shard size per core
    N2: int = field(init=False)  # padded node count (multiple of C*P)
    NG: int = field(init=False)  # total write groups (128 nodes each)
    NBK: int = field(init=False)  # gather buckets
    BSZ: list = field(init=False)  # rows per bucket
    GPB: list = field(init=False)  # groups per bucket
    TPSW: int = field(init=False)  # dst tiles per superwindow
    NSW: int = field(init=False)  # superwindows per core

    def __post_init__(self):
        assert self.BKCAP % P == 0 and self.BKCAP <= 32767
        assert self.SWD % P == 0
        assert self.N % self.C == 0
        self.NS = self.N // self.C
        blk = self.C * P
        self.N2 = ((self.N + blk - 1) // blk) * blk
        self.NG = self.N2 // P
        self.NBK = (self.N2 + self.BKCAP - 1) // self.BKCAP
        self.BSZ = [
            min(self.BKCAP, self.N2 - b * self.BKCAP) for b in range(self.NBK)
        ]
        self.GPB = [sz // P for sz in self.BSZ]
        self.TPSW = self.SWD // P
        self.NSW = (self.NS + self.SWD - 1) // self.SWD


def _wrap_idx(vals16):
    """[K] int16 (K % 128 == 0) -> [128, K//16] wrapped+replicated layout."""
    k = vals16.shape[0]
    w16 = vals16.reshape(k // 16, 16).T  # [16, K/16]
    return np.tile(w16, (8, 1))  # [128, K/16]


@dataclass
class CallMeta:
    sw: int
    bk: int
    k: int  # call index within its (sw, bk) section
    S: int  # slices in this call (one dma_gather per call)
    icol: int  # column offset into gidx array (8 * slice offset)
    scol: int  # column offset into dstl array (slice offset)


def _schedule(prm, n_sl_u):
    """Uniform (core-independent) schedule from the padded slice counts.

    Each (sw, bk) section is chunked into gather calls of <= S_CAP slices.
    Matmuls are emitted bucket-major per sw so PE starts as soon as bucket
    0's gather lands; each dst-tile t accumulates into its own PSUM tensor
    (accumulation groups stay open across buckets).
    Returns (calls, mms_by_sw).
    mms_by_sw[sw] = list of (bk, s_in_section, t, start, stop).
    """
    calls = []
    mms_by_sw = []
    icol = 0
    scol = 0
    for sw in range(prm.NSW):
        for bk in range(prm.NBK):
            nsl = sum(int(n_sl_u[sw][bk][t]) for t in range(prm.TPSW))
            for k, a in enumerate(range(0, nsl, prm.S_CAP)):
                S = min(prm.S_CAP, nsl - a)
                calls.append(CallMeta(sw, bk, k, S, icol, scol))
                icol += 8 * S
                scol += S
        # t-major: each dst-tile's PSUM accumulation group opens and closes
        # before the next opens (a start=True clears its whole PSUM bank, so
        # groups must not interleave within a bank)
        mms = []
        for t in range(prm.TPSW):
            tot = sum(int(n_sl_u[sw][bk][t]) for bk in range(prm.NBK))
            ms = []
            seen = 0
            for bk in range(prm.NBK):
                s0 = sum(int(n_sl_u[sw][bk][tt]) for tt in range(t))
                for _ in range(int(n_sl_u[sw][bk][t])):
                    ms.append((bk, s0, seen == 0, seen == tot - 1))
                    seen += 1
                    s0 += 1
            ms and None
            mms.append(ms)
        mms_by_sw.append(mms)
    return calls, mms_by_sw


def _host_prep(x, edge_index, W, prm):
    N, C, NS = prm.N, prm.C, prm.NS
    src = np.asarray(edge_index[0], dtype=np.int64).astype(np.int32)
    dst = np.asarray(edge_index[1], dtype=np.int64).astype(np.int32)
    x = np.asarray(x, dtype=np.float32)
    W = np.asarray(W, dtype=np.float32)

    deg = np.bincount(dst, minlength=N).astype(np.float64)
    dinv = np.where(deg > 0, 1.0 / np.sqrt(np.maximum(deg, 1.0)), 0.0).astype(
        np.float32
    )

    # table slot of node n: groups are written in blocks of QG=8 with the
    # partition dim outermost so phase-A table writes are contiguous 2KB
    # per partition: slot = (g//8)*1024 + (n%128)*8 + g%8,  g = n//128
    g_e = src // P
    p_e = src % P
    slot_e = (g_e // 8) * (8 * P) + p_e * 8 + (g_e % 8)
    bk_e = (slot_e // prm.BKCAP).astype(np.int32)
    rel_e = (slot_e % prm.BKCAP).astype(np.int16)

    # per-edge attributes
    core_e = dst // NS
    edl = dst - core_e * NS
    sw_e = edl // prm.SWD
    t_e = (edl % prm.SWD) // P
    q_e = (edl % P).astype(np.float32)

    # per-core sorted cell structure
    ncell = prm.NSW * prm.NBK * prm.TPSW
    counts = np.zeros((C, ncell), dtype=np.int64)
    percore = []
    for c in range(C):
        m = core_e == c
        order = np.lexsort((edl[m], t_e[m], bk_e[m], sw_e[m]))
        cell = (sw_e[m] * prm.NBK + bk_e[m]) * prm.TPSW + t_e[m]
        counts[c] = np.bincount(cell, minlength=ncell)
        percore.append(
            {
                "rel": rel_e[m][order],
                "q": q_e[m][order],
                "cell": cell[order],
            }
        )

    # uniform slice counts, >= 1 for in-range (sw, t) on bucket 0
    n_sl_u = np.zeros((prm.NSW, prm.NBK, prm.TPSW), dtype=np.int64)
    cmax = counts.max(axis=0).reshape(prm.NSW, prm.NBK, prm.TPSW)
    n_sl_u[:] = (cmax + P - 1) // P
    for sw in range(prm.NSW):
        ntile = min(prm.TPSW, max(0, -(-(NS - sw * prm.SWD) // P)))
        for t in range(ntile):
            if n_sl_u[sw, :, t].sum() == 0:
                n_sl_u[sw, 0, t] = 1

    calls, mms_by_sw = _schedule(prm, n_sl_u)
    icols = sum(8 * cm.S for cm in calls)
    scols = sum(cm.S for cm in calls)

    # slot offset (in slices) of each cell in the uniform stream
    cell_sl = n_sl_u.reshape(ncell)
    cell_off = np.zeros(ncell, dtype=np.int64)
    np.cumsum(cell_sl[:-1], out=cell_off[1:])
    S_total = int(cell_sl.sum())

    # fill per-core gather-index / dst-local arrays
    gidx_all = np.zeros((C, P, icols), dtype=np.int16)
    dstl_all = np.full((C, P, scols), -1.0, dtype=BF16)
    for c in range(C):
        pc = percore[c]
        ne = pc["cell"].shape[0]
        cc = counts[c]
        starts = np.zeros(ncell, dtype=np.int64)
        np.cumsum(cc[:-1], out=starts[1:])
        rank = np.arange(ne, dtype=np.int64) - starts[pc["cell"]]
        pos = cell_off[pc["cell"]] * P + rank  # global slot position
        vals = np.zeros(S_total * P, dtype=np.int16)
        dvals = np.full(S_total * P, -1.0, dtype=np.float32)
        vals[pos] = pc["rel"]
        dvals[pos] = pc["q"]
        for cm in calls:
            sl0 = cm.scol
            seg = vals[sl0 * P : (sl0 + cm.S) * P]
            gidx_all[c, :, cm.icol : cm.icol + 8 * cm.S] = _wrap_idx(seg)
            dstl_all[c, :, cm.scol : cm.scol + cm.S] = (
                dvals[sl0 * P : (sl0 + cm.S) * P].reshape(cm.S, P).T
            ).astype(BF16)

    # phase-A input: full dinv-scaled x, transposed, bf16 (same on all cores)
    xpad = np.zeros((prm.N2, FIN), dtype=np.float32)
    xpad[:N] = x * dinv[:, None]
    xT = np.ascontiguousarray(xpad.T).astype(BF16)  # [FIN, N2]
    WT = np.ascontiguousarray(W.T).astype(BF16)  # [FIN, FOUT]
    iota = (
        np.broadcast_to(np.arange(P, dtype=np.float32)[None, :], (P, P))
        .astype(BF16)
        .copy()
    )
    dinvD = np.zeros((C, P, prm.NSW * prm.TPSW), dtype=np.float32)
    w_idx = np.arange(prm.NSW * prm.TPSW)
    for c in range(C):
        node = c * NS + w_idx[:, None] * P + np.arange(P)[None, :]
        ok = node < (c + 1) * NS
        dv = np.where(ok, dinv[np.minimum(node, N - 1)], 0.0)
        dinvD[c][np.arange(P)[None, :], w_idx[:, None]] = dv

    inputs = []
    for c in range(C):
        inputs.append(
            {
                "xT": xT,
                "WT": WT,
                "iota": iota,
                "dinvD": dinvD[c],
                "gidx": gidx_all[c],
                "dstl": dstl_all[c],
            }
        )
    return inputs, calls, mms_by_sw, icols, scols


def _split_sync_waits(nc):
    """This env's walrus rejects >1 sync wait on some opcodes; keep 1 wait
    per instruction, moving extras onto preceding same-engine NOPs."""
    for bb in nc.main_func.blocks:
        insts = bb.instructions
        i = 0
        while i < len(insts):
            ins = insts[i]
            si = ins.sync_info
            if si is not None and si.on_wait is not None and len(si.on_wait) > 1:
                waits = list(si.on_wait)
                keep, extra = waits[-1:], waits[:-1]
                k = 0
                while extra:
                    chunk, extra = extra[:1], extra[1:]
                    nop = mybir.InstNoOp(name=f"{ins.name}-ws{k}", ins=[], outs=[])
                    nop.engine = ins.engine
                    nop.sync_info = mybir.SyncInfo(on_wait=chunk, on_update=[])
                    nc.register_instruction(nop)
                    insts.insert(i, nop)
                    i += 1
                    k += 1
                ins.sync_info = mybir.SyncInfo(
                    on_wait=keep, on_update=list(si.on_update or [])
                )
            i += 1


def _build_program(prm, calls, mms_by_sw, icols, scols):
    f32 = mybir.dt.float32
    bf16 = mybir.dt.bfloat16
    nc = bacc.Bacc("TRN2", num_swdge_queues=4)

    xT = nc.declare_dram_parameter("xT", [FIN, prm.N2], bf16, isOutput=False)
    WT = nc.declare_dram_parameter("WT", [FIN, FOUT], bf16, isOutput=False)
    iota = nc.declare_dram_parameter("iota", [P, P], bf16, isOutput=False)
    dinvD = nc.declare_dram_parameter(
        "dinvD", [P, prm.NSW * prm.TPSW], f32, isOutput=False
    )
    gidx = nc.declare_dram_parameter(
        "gidx", [P, icols], mybir.dt.int16, isOutput=False
    )
    dstl = nc.declare_dram_parameter("dstl", [P, scols], bf16, isOutput=False)
    y = nc.declare_dram_parameter("y", [prm.NS, FOUT], f32, isOutput=True)
    # the full table, replicated per core, split per gather bucket so phase
    # B's bucket-b gathers only wait on bucket b's writes
    TBL = [
        nc.dram_tensor(f"tbl{b}", [prm.BSZ[b], ROWB], bf16)
        for b in range(prm.NBK)
    ]

    with tile.TileContext(nc) as tc:
        with tc.tile_pool(name="const", bufs=1) as cpool:
            wt_sb = cpool.tile([FIN, FOUT], bf16, tag="wt")
            nc.sync.dma_start(out=wt_sb[:], in_=WT[:])
            iota_sb = cpool.tile([P, P], bf16, tag="io")
            nc.sync.dma_start(out=iota_sb[:], in_=iota[:])
            dinvD_sb = cpool.tile([P, prm.NSW * prm.TPSW], f32, tag="dd")
            nc.sync.dma_start(out=dinvD_sb[:], in_=dinvD[:])
            # preload all dst-lane ids (small); gather indices are staged
            # per-superwindow below
            dstl_sb = cpool.tile([P, scols], bf16, tag="dl")
            nc.scalar.dma_start(out=dstl_sb[:], in_=dstl[:])

            # ---------------- Phase A: build the full h~ table ------------
            QG = 8  # groups per PSUM tile / activation / q-block (1024 rows)
            LB = 4  # q-blocks per load & table write (1MB DMAs: the ~2us
            #         fixed cost per dma_start dominates small ones)
            assert prm.NG % QG == 0
            NQB = prm.NG // QG  # 1024-row q-blocks
            with (
                tc.tile_pool(name="pa", bufs=3) as pa,
                tc.tile_pool(name="pat", bufs=3) as pat,
                tc.tile_pool(name="psa", bufs=4, space="PSUM") as psa,
            ):
                for q0 in range(0, NQB, LB):
                    q1 = min(q0 + LB, NQB)
                    nq = q1 - q0
                    xt = pa.tile([P, LB * QG * P], bf16, tag="xt")
                    nc.scalar.dma_start(
                        out=xt[:, : nq * QG * P],
                        in_=xT[:, q0 * QG * P : q1 * QG * P],
                    )
                    tsb = pat.tile([P, LB, QG, ROWB], bf16, tag="tsb")
                    for q in range(nq):
                        hps = psa.tile([P, QG, FOUT], f32, tag="hps")
                        for j in range(QG):
                            nc.tensor.matmul(
                                out=hps[:, j, :],
                                lhsT=xt[:, (q * QG + j) * P : (q * QG + j + 1) * P],
                                rhs=wt_sb[:],
                                start=True,
                                stop=True,
                            )
                        nc.scalar.activation(
                            out=tsb[:, q, :, :FOUT],
                            in_=hps[:],
                            func=mybir.ActivationFunctionType.Copy,
                        )
                    # block rows are laid out (p, j): partition-major, so the
                    # write is 2KB contiguous per partition per q-block; a
                    # write is split where it straddles a bucket boundary
                    qa = q0
                    while qa < q1:
                        b = (qa * QG * P) // prm.BKCAP
                        qb_lim = ((b + 1) * prm.BKCAP) // (QG * P)
                        qb = min(q1, qb_lim)
                        r0 = qa * QG * P - b * prm.BKCAP
                        nc.sync.dma_start(
                            out=TBL[b][r0 : r0 + (qb - qa) * QG * P, :].rearrange(
                                "(q p j) f -> p q j f", j=QG, p=P
                            ),
                            in_=tsb[:, qa - q0 : qb - q0, :, :],
                        )
                        qa = qb

            # ---------------- Phase B: gather + segment-sum ----------------
            S_MAX = max((cm.S for cm in calls), default=1)
            calls_by_sw = [[] for _ in range(prm.NSW)]
            for cm in calls:
                calls_by_sw[cm.sw].append(cm)
            # slices per (sw, bk) section and its slice/scol offsets
            sec_nsl = {}
            sec_scol = {}
            for cm in calls:
                key = (cm.sw, cm.bk)
                if key not in sec_nsl:
                    sec_nsl[key] = 0
                    sec_scol[key] = cm.scol
                sec_nsl[key] += cm.S
            SB_MAX = max(sec_nsl.values(), default=1)
            # per-sw gidx column spans (calls of a sw are contiguous in icol)
            sw_icol = [
                (cs[0].icol, cs[-1].icol + 8 * cs[-1].S) if cs else (0, 0)
                for cs in calls_by_sw
            ]
            ICW_MAX = max((b - a for a, b in sw_icol), default=1)
            qctr = [0]
            with (
                tc.tile_pool(name="pgi", bufs=4) as pgi,
                tc.tile_pool(name="pg", bufs=32) as pg,
                tc.tile_pool(name="pb", bufs=4) as pb,
                tc.tile_pool(name="py", bufs=8) as py,
                tc.tile_pool(name="psb", bufs=6, space="PSUM") as psb,
            ):
                for sw in range(prm.NSW):
                    if not calls_by_sw[sw]:
                        continue
                    ic0, ic1 = sw_icol[sw]
                    gsw = pgi.tile([P, ICW_MAX], mybir.dt.int16, tag="gi")
                    nc.scalar.dma_start(
                        out=gsw[:, : ic1 - ic0], in_=gidx[:, ic0:ic1]
                    )
                    # build the whole sw's one-hot B sections up front so DVE
                    # never gates the PE mid-superwindow
                    bsec = {}
                    for bk in range(prm.NBK):
                        if (sw, bk) not in sec_nsl:
                            continue
                        ns = sec_nsl[(sw, bk)]
                        sc = sec_scol[(sw, bk)]
                        b_t = pb.tile([P, SB_MAX, P], bf16, tag="b")
                        nc.vector.tensor_tensor(
                            out=b_t[:, :ns, :],
                            in0=dstl_sb[:, sc : sc + ns][
                                :, :, None
                            ].to_broadcast([P, ns, P]),
                            in1=iota_sb[:, None, :].to_broadcast([P, ns, P]),
                            op=mybir.AluOpType.is_equal,
                        )
                        bsec[bk] = b_t
                    tiles = {}  # (bk, k) -> g_t
                    for cm in calls_by_sw[sw]:
                        S = cm.S
                        g_t = pg.tile([P, S_MAX, ROWB], bf16, tag="g")
                        nc.gpsimd.dma_gather(
                            out_ap=g_t[:, :S, :],
                            in_ap=TBL[cm.bk][:],
                            idxs_ap=gsw[:, cm.icol - ic0 : cm.icol - ic0 + 8 * S],
                            num_idxs=S * P,
                            num_idxs_reg=S * P,
                            elem_size=ROWB,
                            single_packet=False,
                            queue_num=qctr[0] % 4,
                        )
                        qctr[0] += 1
                        tiles[(cm.bk, cm.k)] = g_t
                    rows_sw = min(prm.SWD, prm.NS - sw * prm.SWD)
                    nt = (rows_sw + P - 1) // P  # valid dst tiles this sw
                    for t in range(prm.TPSW):
                        if not mms_by_sw[sw][t]:
                            continue
                        psum_t = psb.tile([P, FOUT], f32, tag="acc")
                        for bk, s, st, sp in mms_by_sw[sw][t]:
                            g_t = tiles[(bk, s // prm.S_CAP)]
                            sl = s % prm.S_CAP
                            nc.tensor.matmul(
                                out=psum_t[:],
                                lhsT=bsec[bk][:, s, :],
                                rhs=g_t[:, sl, :FOUT],
                                start=st,
                                stop=sp,
                            )
                        if t >= nt:
                            continue
                        # scale by dinv[dst] on the otherwise-idle Scalar
                        # engine, then land this dst tile of y
                        w = sw * prm.TPSW + t
                        ysb = py.tile([P, FOUT], f32, tag="ysb")
                        nc.scalar.activation(
                            out=ysb[:],
                            in_=psum_t[:],
                            func=mybir.ActivationFunctionType.Copy,
                            scale=dinvD_sb[:, w : w + 1],
                        )
                        rt = min(P, rows_sw - t * P)
                        r0 = sw * prm.SWD + t * P
                        nc.sync.dma_start(out=y[r0 : r0 + rt, :], in_=ysb[:rt, :])

    nc.compile()
    _split_sync_waits(nc)
    return nc


def _get_program_and_prep(x, edge_index, W, prm):
    inputs, calls, mms_by_sw, icols, scols = _host_prep(x, edge_index, W, prm)
    nc = _build_program(prm, calls, mms_by_sw, icols, scols)
    return nc, inputs


def kernel(x, edge_index, W):
    prm = Prm(N=int(x.shape[0]))
    nc, inputs = _get_program_and_prep(x, edge_index, W, prm)
    res = run_bass_kernel_spmd(nc, inputs, list(range(prm.C)))
    y = np.concatenate([res.results[c]["y"] for c in range(prm.C)], axis=0)
    return y.astype(np.float32)


def run_with_trace(x, edge_index, W, trace_cores=None):
    """test.py helper: returns (y, BassKernelResults) with profiling."""
    prm = Prm(N=int(x.shape[0]))
    nc, inputs = _get_program_and_prep(x, edge_index, W, prm)
    res = run_bass_kernel_spmd(
        nc, inputs, list(range(prm.C)), trace=True, trace_cores=trace_cores
    )
    y = np.concatenate([res.results[c]["y"] for c in range(prm.C)], axis=0)
    return y.astype(np.float32), res


# revision 22
# speedup vs baseline: 1.1089x; 1.0506x over previous
"""GCN inference kernel (y = D^-1/2 A D^-1/2 (x @ W.T)) on 8 Trainium2 NeuronCores.

Strategy (full inputs in, full output out; sharded internally):
  - Destination nodes are sharded across the 8 cores (12500 dsts each);
    edges are owned by the core that owns their dst, so the segment-sum is
    core-local (per the sharding hint).
  - Phase A (replicated): every core computes the full scaled projection
    table h~[n] = (dinv[n]*x[n]) @ W.T in bf16 with PE matmuls (dinv is
    folded into x host-side) and writes it to per-bucket HBM tables; rows
    are 256B (64 bf16 features + 64 bf16 pad, never read).  No collective:
    phase B's bucket-b gathers start as soon as bucket b's rows land.
  - Phase B (per core): SWDGE dma_gather streams h~[src] rows (256B each)
    for the core's dst-sorted edge list into SBUF; a one-hot selection
    matrix B (built on DVE in bf16 from dst-local ids vs an iota row)
    turns the segment-sum into bf16 PE matmuls accumulated in PSUM per
    128-dst tile; a final per-dst dinv scale (Scalar engine) lands y.
    All gather indices / dst-lane ids are preloaded into SBUF once.
  - All data-dependent structure (edge sort, padding, gather indices,
    one-hot ids, uniform per-core slice schedule) is prepared host-side in
    numpy; the device program is identical on all 8 cores (SPMD), only the
    per-core input arrays differ.
"""

from dataclasses import dataclass, field

import numpy as np
import ml_dtypes

import concourse.bacc as bacc
import concourse.mybir as mybir
import concourse.tile as tile
from concourse.bass_utils import run_bass_kernel_spmd

P = 128  # SBUF partitions
FIN = 128
FOUT = 64
ROWB = 128  # padded table row width (bf16 -> 256B rows for dma_gather)

BF16 = ml_dtypes.bfloat16


@dataclass
class Prm:
    N: int = 100000  # nodes
    C: int = 8  # cores
    BKCAP: int = 25600  # table rows per gather bucket (int16 idx limit)
    SWD: int = 512  # dst nodes per superwindow (TPSW * P)
    S_CAP: int = 12  # max slices per dma_gather call (pipelining granularity)
    NS: int = field(init=False)  # dst shard size per core
    N2: int = field(init=False)  # padded node count (multiple of C*P)
    NG: int = field(init=False)  # total write groups (128 nodes each)
    NBK: int = field(init=False)  # gather buckets
    BSZ: list = field(init=False)  # rows per bucket
    GPB: list = field(init=False)  # groups per bucket
    TPSW: int = field(init=False)  # dst tiles per superwindow
    NSW: int = field(init=False)  # superwindows per core

    def __post_init__(self):
        assert self.BKCAP % P == 0 and self.BKCAP <= 32767
        assert self.SWD % P == 0
        assert self.N % self.C == 0
        self.NS = self.N // self.C
        blk = self.C * P
        self.N2 = ((self.N + blk - 1) // blk) * blk
        self.NG = self.N2 // P
        self.NBK = (self.N2 + self.BKCAP - 1) // self.BKCAP
        self.BSZ = [
            min(self.BKCAP, self.N2 - b * self.BKCAP) for b in range(self.NBK)
        ]
        self.GPB = [sz // P for sz in self.BSZ]
        self.TPSW = self.SWD // P
        self.NSW = (self.NS + self.SWD - 1) // self.SWD


def _wrap_idx(vals16):
    """[K] int16 (K % 128 == 0) -> [128, K//16] wrapped+replicated layout."""
    k = vals16.shape[0]
    w16 = vals16.reshape(k // 16, 16).T  # [16, K/16]
    return np.tile(w16, (8, 1))  # [128, K/16]


@dataclass
class CallMeta:
    sw: int
    bk: int
    k: int  # call index within its (sw, bk) section
    S: int  # slices in this call (one dma_gather per call)
    icol: int  # column offset into gidx array (8 * slice offset)
    scol: int  # column offset into dstl array (slice offset)


def _schedule(prm, n_sl_u):
    """Uniform (core-independent) schedule from the padded slice counts.

    Each (sw, bk) section is chunked into gather calls of <= S_CAP slices.
    Matmuls are emitted bucket-major per sw so PE starts as soon as bucket
    0's gather lands; each dst-tile t accumulates into its own PSUM tensor
    (accumulation groups stay open across buckets).
    Returns (calls, mms_by_sw).
    mms_by_sw[sw] = list of (bk, s_in_section, t, start, stop).
    """
    calls = []
    mms_by_sw = []
    icol = 0
    scol = 0
    for sw in range(prm.NSW):
        for bk in range(prm.NBK):
            nsl = sum(int(n_sl_u[sw][bk][t]) for t in range(prm.TPSW))
            for k, a in enumerate(range(0, nsl, prm.S_CAP)):
                S = min(prm.S_CAP, nsl - a)
                calls.append(CallMeta(sw, bk, k, S, icol, scol))
                icol += 8 * S
                scol += S
        # t-major: each dst-tile's PSUM accumulation group opens and closes
        # before the next opens (a start=True clears its whole PSUM bank, so
        # groups must not interleave within a bank)
        mms = []
        for t in range(prm.TPSW):
            tot = sum(int(n_sl_u[sw][bk][t]) for bk in range(prm.NBK))
            ms = []
            seen = 0
            for bk in range(prm.NBK):
                s0 = sum(int(n_sl_u[sw][bk][tt]) for tt in range(t))
                for _ in range(int(n_sl_u[sw][bk][t])):
                    ms.append((bk, s0, seen == 0, seen == tot - 1))
                    seen += 1
                    s0 += 1
            ms and None
            mms.append(ms)
        mms_by_sw.append(mms)
    return calls, mms_by_sw


def _host_prep(x, edge_index, W, prm):
    N, C, NS = prm.N, prm.C, prm.NS
    src = np.asarray(edge_index[0], dtype=np.int64).astype(np.int32)
    dst = np.asarray(edge_index[1], dtype=np.int64).astype(np.int32)
    x = np.asarray(x, dtype=np.float32)
    W = np.asarray(W, dtype=np.float32)

    deg = np.bincount(dst, minlength=N).astype(np.float64)
    dinv = np.where(deg > 0, 1.0 / np.sqrt(np.maximum(deg, 1.0)), 0.0).astype(
        np.float32
    )

    # table slot of node n: groups are written in blocks of QG=8 with the
    # partition dim outermost so phase-A table writes are contiguous 2KB
    # per partition: slot = (g//8)*1024 + (n%128)*8 + g%8,  g = n//128
    g_e = src // P
    p_e = src % P
    slot_e = (g_e // 8) * (8 * P) + p_e * 8 + (g_e % 8)
    bk_e = (slot_e // prm.BKCAP).astype(np.int32)
    rel_e = (slot_e % prm.BKCAP).astype(np.int16)

    # per-edge attributes
    core_e = dst // NS
    edl = dst - core_e * NS
    sw_e = edl // prm.SWD
    t_e = (edl % prm.SWD) // P
    q_e = (edl % P).astype(np.float32)

    # per-core sorted cell structure
    ncell = prm.NSW * prm.NBK * prm.TPSW
    counts = np.zeros((C, ncell), dtype=np.int64)
    percore = []
    for c in range(C):
        m = core_e == c
        order = np.lexsort((edl[m], t_e[m], bk_e[m], sw_e[m]))
        cell = (sw_e[m] * prm.NBK + bk_e[m]) * prm.TPSW + t_e[m]
        counts[c] = np.bincount(cell, minlength=ncell)
        percore.append(
            {
                "rel": rel_e[m][order],
                "q": q_e[m][order],
                "cell": cell[order],
            }
        )

    # uniform slice counts, >= 1 for in-range (sw, t) on bucket 0
    n_sl_u = np.zeros((prm.NSW, prm.NBK, prm.TPSW), dtype=np.int64)
    cmax = counts.max(axis=0).reshape(prm.NSW, prm.NBK, prm.TPSW)
    n_sl_u[:] = (cmax + P - 1) // P
    for sw in range(prm.NSW):
        ntile = min(prm.TPSW, max(0, -(-(NS - sw * prm.SWD) // P)))
        for t in range(ntile):
            if n_sl_u[sw, :, t].sum() == 0:
                n_sl_u[sw, 0, t] = 1

    calls, mms_by_sw = _schedule(prm, n_sl_u)
    icols = sum(8 * cm.S for cm in calls)
    scols = sum(cm.S for cm in calls)

    # slot offset (in slices) of each cell in the uniform stream
    cell_sl = n_sl_u.reshape(ncell)
    cell_off = np.zeros(ncell, dtype=np.int64)
    np.cumsum(cell_sl[:-1], out=cell_off[1:])
    S_total = int(cell_sl.sum())

    # fill per-core gather-index / dst-local arrays
    gidx_all = np.zeros((C, P, icols), dtype=np.int16)
    dstl_all = np.full((C, P, scols), -1.0, dtype=BF16)
    for c in range(C):
        pc = percore[c]
        ne = pc["cell"].shape[0]
        cc = counts[c]
        starts = np.zeros(ncell, dtype=np.int64)
        np.cumsum(cc[:-1], out=starts[1:])
        rank = np.arange(ne, dtype=np.int64) - starts[pc["cell"]]
        pos = cell_off[pc["cell"]] * P + rank  # global slot position
        vals = np.zeros(S_total * P, dtype=np.int16)
        dvals = np.full(S_total * P, -1.0, dtype=np.float32)
        vals[pos] = pc["rel"]
        dvals[pos] = pc["q"]
        for cm in calls:
            sl0 = cm.scol
            seg = vals[sl0 * P : (sl0 + cm.S) * P]
            gidx_all[c, :, cm.icol : cm.icol + 8 * cm.S] = _wrap_idx(seg)
            dstl_all[c, :, cm.scol : cm.scol + cm.S] = (
                dvals[sl0 * P : (sl0 + cm.S) * P].reshape(cm.S, P).T
            ).astype(BF16)

    # phase-A input: full dinv-scaled x, transposed, bf16 (same on all cores)
    xpad = np.zeros((prm.N2, FIN), dtype=np.float32)
    xpad[:N] = x * dinv[:, None]
    xT = np.ascontiguousarray(xpad.T).astype(BF16)  # [FIN, N2]
    WT = np.ascontiguousarray(W.T).astype(BF16)  # [FIN, FOUT]
    iota = (
        np.broadcast_to(np.arange(P, dtype=np.float32)[None, :], (P, P))
        .astype(BF16)
        .copy()
    )
    dinvD = np.zeros((C, P, prm.NSW * prm.TPSW), dtype=np.float32)
    w_idx = np.arange(prm.NSW * prm.TPSW)
    for c in range(C):
        node = c * NS + w_idx[:, None] * P + np.arange(P)[None, :]
        ok = node < (c + 1) * NS
        dv = np.where(ok, dinv[np.minimum(node, N - 1)], 0.0)
        dinvD[c][np.arange(P)[None, :], w_idx[:, None]] = dv

    inputs = []
    for c in range(C):
        inputs.append(
            {
                "xT": xT,
                "WT": WT,
                "iota": iota,
                "dinvD": dinvD[c],
                "gidx": gidx_all[c],
                "dstl": dstl_all[c],
            }
        )
    return inputs, calls, mms_by_sw, icols, scols


def _split_sync_waits(nc):
    """This env's walrus rejects >1 sync wait on some opcodes; keep 1 wait
    per instruction, moving extras onto preceding same-engine NOPs."""
    for bb in nc.main_func.blocks:
        insts = bb.instructions
        i = 0
        while i < len(insts):
            ins = insts[i]
            si = ins.sync_info
            if si is not None and si.on_wait is not None and len(si.on_wait) > 1:
                waits = list(si.on_wait)
                keep, extra = waits[-1:], waits[:-1]
                k = 0
                while extra:
                    chunk, extra = extra[:1], extra[1:]
                    nop = mybir.InstNoOp(name=f"{ins.name}-ws{k}", ins=[], outs=[])
                    nop.engine = ins.engine
                    nop.sync_info = mybir.SyncInfo(on_wait=chunk, on_update=[])
                    nc.register_instruction(nop)
                    insts.insert(i, nop)
                    i += 1
                    k += 1
                ins.sync_info = mybir.SyncInfo(
                    on_wait=keep, on_update=list(si.on_update or [])
                )
            i += 1


def _build_program(prm, calls, mms_by_sw, icols, scols):
    f32 = mybir.dt.float32
    bf16 = mybir.dt.bfloat16
    nc = bacc.Bacc("TRN2", num_swdge_queues=4, dynamic_dma_scratch_size=49152)

    xT = nc.declare_dram_parameter("xT", [FIN, prm.N2], bf16, isOutput=False)
    WT = nc.declare_dram_parameter("WT", [FIN, FOUT], bf16, isOutput=False)
    iota = nc.declare_dram_parameter("iota", [P, P], bf16, isOutput=False)
    dinvD = nc.declare_dram_parameter(
        "dinvD", [P, prm.NSW * prm.TPSW], f32, isOutput=False
    )
    gidx = nc.declare_dram_parameter(
        "gidx", [P, icols], mybir.dt.int16, isOutput=False
    )
    dstl = nc.declare_dram_parameter("dstl", [P, scols], bf16, isOutput=False)
    y = nc.declare_dram_parameter("y", [prm.NS, FOUT], f32, isOutput=True)
    # the full table, replicated per core, split per gather bucket so phase
    # B's bucket-b gathers only wait on bucket b's writes
    TBL = [
        nc.dram_tensor(f"tbl{b}", [prm.BSZ[b], ROWB], bf16)
        for b in range(prm.NBK)
    ]

    with tile.TileContext(nc) as tc:
        with tc.tile_pool(name="const", bufs=1) as cpool:
            wt_sb = cpool.tile([FIN, FOUT], bf16, tag="wt")
            nc.sync.dma_start(out=wt_sb[:], in_=WT[:])
            iota_sb = cpool.tile([P, P], bf16, tag="io")
            nc.sync.dma_start(out=iota_sb[:], in_=iota[:])
            dinvD_sb = cpool.tile([P, prm.NSW * prm.TPSW], f32, tag="dd")
            nc.sync.dma_start(out=dinvD_sb[:], in_=dinvD[:])
            # preload all dst-lane ids (small); gather indices are staged
            # per-superwindow below
            dstl_sb = cpool.tile([P, scols], bf16, tag="dl")
            nc.scalar.dma_start(out=dstl_sb[:], in_=dstl[:])

            # ---------------- Phase A: build the full h~ table ------------
            QG = 8  # groups per PSUM tile / activation / q-block (1024 rows)
            LB = 4  # q-blocks per load & table write (1MB DMAs: the ~2us
            #         fixed cost per dma_start dominates small ones)
            assert prm.NG % QG == 0
            NQB = prm.NG // QG  # 1024-row q-blocks
            with (
                tc.tile_pool(name="pa", bufs=3) as pa,
                tc.tile_pool(name="pat", bufs=3) as pat,
                tc.tile_pool(name="psa", bufs=4, space="PSUM") as psa,
            ):
                for q0 in range(0, NQB, LB):
                    q1 = min(q0 + LB, NQB)
                    nq = q1 - q0
                    xt = pa.tile([P, LB * QG * P], bf16, tag="xt")
                    nc.scalar.dma_start(
                        out=xt[:, : nq * QG * P],
                        in_=xT[:, q0 * QG * P : q1 * QG * P],
                    )
                    tsb = pat.tile([P, LB, QG, ROWB], bf16, tag="tsb")
                    for q in range(nq):
                        hps = psa.tile([P, QG, FOUT], f32, tag="hps")
                        for j in range(QG):
                            nc.tensor.matmul(
                                out=hps[:, j, :],
                                lhsT=xt[:, (q * QG + j) * P : (q * QG + j + 1) * P],
                                rhs=wt_sb[:],
                                start=True,
                                stop=True,
                            )
                        nc.scalar.activation(
                            out=tsb[:, q, :, :FOUT],
                            in_=hps[:],
                            func=mybir.ActivationFunctionType.Copy,
                        )
                    # block rows are laid out (p, j): partition-major, so the
                    # write is 2KB contiguous per partition per q-block; a
                    # write is split where it straddles a bucket boundary
                    qa = q0
                    while qa < q1:
                        b = (qa * QG * P) // prm.BKCAP
                        qb_lim = ((b + 1) * prm.BKCAP) // (QG * P)
                        qb = min(q1, qb_lim)
                        r0 = qa * QG * P - b * prm.BKCAP
                        nc.sync.dma_start(
                            out=TBL[b][r0 : r0 + (qb - qa) * QG * P, :].rearrange(
                                "(q p j) f -> p q j f", j=QG, p=P
                            ),
                            in_=tsb[:, qa - q0 : qb - q0, :, :],
                        )
                        qa = qb

            # ---------------- Phase B: gather + segment-sum ----------------
            S_MAX = max((cm.S for cm in calls), default=1)
            calls_by_sw = [[] for _ in range(prm.NSW)]
            for cm in calls:
                calls_by_sw[cm.sw].append(cm)
            # slices per (sw, bk) section and its slice/scol offsets
            sec_nsl = {}
            sec_scol = {}
            for cm in calls:
                key = (cm.sw, cm.bk)
                if key not in sec_nsl:
                    sec_nsl[key] = 0
                    sec_scol[key] = cm.scol
                sec_nsl[key] += cm.S
            SB_MAX = max(sec_nsl.values(), default=1)
            # per-sw gidx column spans (calls of a sw are contiguous in icol)
            sw_icol = [
                (cs[0].icol, cs[-1].icol + 8 * cs[-1].S) if cs else (0, 0)
                for cs in calls_by_sw
            ]
            ICW_MAX = max((b - a for a, b in sw_icol), default=1)
            qctr = [0]
            with (
                tc.tile_pool(name="pgi", bufs=4) as pgi,
                tc.tile_pool(name="pg", bufs=32) as pg,
                tc.tile_pool(name="pb", bufs=4) as pb,
                tc.tile_pool(name="py", bufs=8) as py,
                tc.tile_pool(name="psb", bufs=6, space="PSUM") as psb,
            ):
                for sw in range(prm.NSW):
                    if not calls_by_sw[sw]:
                        continue
                    ic0, ic1 = sw_icol[sw]
                    gsw = pgi.tile([P, ICW_MAX], mybir.dt.int16, tag="gi")
                    nc.scalar.dma_start(
                        out=gsw[:, : ic1 - ic0], in_=gidx[:, ic0:ic1]
                    )
                    # build the whole sw's one-hot B sections up front so DVE
                    # never gates the PE mid-superwindow
                    bsec = {}
                    for bk in range(prm.NBK):
                        if (sw, bk) not in sec_nsl:
                            continue
                        ns = sec_nsl[(sw, bk)]
                        sc = sec_scol[(sw, bk)]
                        b_t = pb.tile([P, SB_MAX, P], bf16, tag="b")
                        nc.vector.tensor_tensor(
                            out=b_t[:, :ns, :],
                            in0=dstl_sb[:, sc : sc + ns][
                                :, :, None
                            ].to_broadcast([P, ns, P]),
                            in1=iota_sb[:, None, :].to_broadcast([P, ns, P]),
                            op=mybir.AluOpType.is_equal,
                        )
                        bsec[bk] = b_t
                    tiles = {}  # (bk, k) -> g_t
                    for cm in calls_by_sw[sw]:
                        S = cm.S
                        g_t = pg.tile([P, S_MAX, ROWB], bf16, tag="g")
                        nc.gpsimd.dma_gather(
                            out_ap=g_t[:, :S, :],
                            in_ap=TBL[cm.bk][:],
                            idxs_ap=gsw[:, cm.icol - ic0 : cm.icol - ic0 + 8 * S],
                            num_idxs=S * P,
                            num_idxs_reg=S * P,
                            elem_size=ROWB,
                            single_packet=False,
                            queue_num=qctr[0] % 4,
                        )
                        qctr[0] += 1
                        tiles[(cm.bk, cm.k)] = g_t
                    rows_sw = min(prm.SWD, prm.NS - sw * prm.SWD)
                    nt = (rows_sw + P - 1) // P  # valid dst tiles this sw
                    for t in range(prm.TPSW):
                        if not mms_by_sw[sw][t]:
                            continue
                        psum_t = psb.tile([P, FOUT], f32, tag="acc")
                        for bk, s, st, sp in mms_by_sw[sw][t]:
                            g_t = tiles[(bk, s // prm.S_CAP)]
                            sl = s % prm.S_CAP
                            nc.tensor.matmul(
                                out=psum_t[:],
                                lhsT=bsec[bk][:, s, :],
                                rhs=g_t[:, sl, :FOUT],
                                start=st,
                                stop=sp,
                            )
                        if t >= nt:
                            continue
                        # scale by dinv[dst] on the otherwise-idle Scalar
                        # engine, then land this dst tile of y
                        w = sw * prm.TPSW + t
                        ysb = py.tile([P, FOUT], f32, tag="ysb")
                        nc.scalar.activation(
                            out=ysb[:],
                            in_=psum_t[:],
                            func=mybir.ActivationFunctionType.Copy,
                            scale=dinvD_sb[:, w : w + 1],
                        )
                        rt = min(P, rows_sw - t * P)
                        r0 = sw * prm.SWD + t * P
                        nc.sync.dma_start(out=y[r0 : r0 + rt, :], in_=ysb[:rt, :])

    nc.compile()
    _split_sync_waits(nc)
    return nc


def _get_program_and_prep(x, edge_index, W, prm):
    inputs, calls, mms_by_sw, icols, scols = _host_prep(x, edge_index, W, prm)
    nc = _build_program(prm, calls, mms_by_sw, icols, scols)
    return nc, inputs


def kernel(x, edge_index, W):
    prm = Prm(N=int(x.shape[0]))
    nc, inputs = _get_program_and_prep(x, edge_index, W, prm)
    res = run_bass_kernel_spmd(nc, inputs, list(range(prm.C)))
    y = np.concatenate([res.results[c]["y"] for c in range(prm.C)], axis=0)
    return y.astype(np.float32)


def run_with_trace(x, edge_index, W, trace_cores=None):
    """test.py helper: returns (y, BassKernelResults) with profiling."""
    prm = Prm(N=int(x.shape[0]))
    nc, inputs = _get_program_and_prep(x, edge_index, W, prm)
    res = run_bass_kernel_spmd(
        nc, inputs, list(range(prm.C)), trace=True, trace_cores=trace_cores
    )
    y = np.concatenate([res.results[c]["y"] for c in range(prm.C)], axis=0)
    return y.astype(np.float32), res
